# revision 9
# baseline (speedup 1.0000x reference)
"""Trainium2 Bass kernel for a dense transformer block (nn_Block_78743930405073).

Block: x -> LN1 -> 16-head causal self-attention -> +x -> LN2 -> FFN(4096, ReLU) -> +.
Input x: [4, 2048, 1024] fp32.  8 NeuronCores, data-parallel over (batch, q-blocks).

Sharding: core c handles batch c//2.  The 16 query-blocks (128 rows each) of a
batch are split between the 2 cores of that batch in an interleaved pattern
(odd blocks / even blocks) so that both cores run the IDENTICAL program (SPMD)
with per-core data: slot j on every core processes one q-block over exactly
2j+2 key-blocks; causality differences between cores are handled by per-core
mask inputs applied to the last two key-blocks of each slot.

Precision: matmuls in bf16 (fp32 PSUM accumulation); LayerNorm statistics,
softmax normalization and the residual stream in fp32.
"""

import sys

if "/opt/trn_rl_repo" not in sys.path:
    sys.path.insert(0, "/opt/trn_rl_repo")

from contextlib import ExitStack

import ml_dtypes
import numpy as np

import concourse.bacc as bacc
import concourse.mybir as mybir
import concourse.tile as tile
from concourse import bass_utils

BF16 = mybir.dt.bfloat16
F32 = mybir.dt.float32
AF = mybir.ActivationFunctionType
AX = mybir.AxisListType

B, T, C = 4, 2048, 1024
NH, HD = 16, 64
FF = 4 * C
EPS = 1e-5
NB = T // 128          # 16 key/query blocks per batch
NSLOT = 8              # q-blocks per core
ROWS = NSLOT * 128     # 1024 own rows per core
NCORES = 8


def _own_blocks(half):
    # half 0 -> odd blocks {1,3,...,15}; half 1 -> even {0,2,...,14}.
    # slot j: trip count Tj = 2j+2 key-blocks on both cores.
    return [2 * j + 1 for j in range(NSLOT)] if half == 0 else [2 * j for j in range(NSLOT)]


def _trip(j):
    return 2 * j + 2


# ---------------------------------------------------------------- bass program


def _ln_tile(nc, pools, xa, h_out):
    """LayerNorm one [128, C] fp32 AP -> h_out [128, C] bf16 (pure normalize)."""
    st = pools["stats"]
    ssum = st.tile([128, 1], F32, tag="ssum")
    ssq = st.tile([128, 1], F32, tag="ssq")
    sq = pools["sq"].tile([128, C], F32, tag="sq")
    nc.vector.reduce_sum(ssum[:], xa, axis=AX.X)
    nc.scalar.activation(sq[:], xa, AF.Square, accum_out=ssq[:])
    mu = st.tile([128, 1], F32, tag="mu")
    ex2 = st.tile([128, 1], F32, tag="ex2")
    var = st.tile([128, 1], F32, tag="var")
    std = st.tile([128, 1], F32, tag="std")
    rstd = st.tile([128, 1], F32, tag="rstd")
    nmr = st.tile([128, 1], F32, tag="nmr")
    nc.vector.tensor_scalar_mul(mu[:], ssum[:], 1.0 / C)
    nc.vector.tensor_scalar_mul(ex2[:], ssq[:], 1.0 / C)
    nc.vector.tensor_mul(var[:], mu[:], mu[:])
    nc.vector.tensor_sub(var[:], ex2[:], var[:])
    nc.vector.tensor_scalar_add(var[:], var[:], EPS)
    nc.scalar.activation(std[:], var[:], AF.Sqrt)
    nc.vector.reciprocal(rstd[:], std[:])
    nc.vector.tensor_mul(nmr[:], mu[:], rstd[:])
    nc.vector.tensor_scalar_mul(nmr[:], nmr[:], -1.0)
    nc.scalar.activation(h_out, xa, AF.Identity, bias=nmr[:], scale=rstd[:])


def build_program():
    nc = bacc.Bacc("TRN2", target_bir_lowering=False, debug=False)

    # ---- DRAM I/O (per-core shapes; identical on all cores) ----
    d = {}
    d["x_full"] = nc.dram_tensor("x_full", [T, C], F32, kind="ExternalInput")
    d["x_own"] = nc.dram_tensor("x_own", [ROWS, C], F32, kind="ExternalInput")
    d["wq"] = nc.dram_tensor("wq", [C, C], BF16, kind="ExternalInput")
    d["wk"] = nc.dram_tensor("wk", [C, C], BF16, kind="ExternalInput")
    d["wv"] = nc.dram_tensor("wv", [C, C], BF16, kind="ExternalInput")
    d["wo"] = nc.dram_tensor("wo", [C + 128, C], BF16, kind="ExternalInput")
    d["w1"] = nc.dram_tensor("w1", [C, FF], BF16, kind="ExternalInput")
    d["w2"] = nc.dram_tensor("w2", [FF + 128, C], BF16, kind="ExternalInput")
    d["bq"] = nc.dram_tensor("bq", [C], F32, kind="ExternalInput")
    d["bk"] = nc.dram_tensor("bk", [C], F32, kind="ExternalInput")
    d["b1"] = nc.dram_tensor("b1", [FF], F32, kind="ExternalInput")
    d["masks"] = nc.dram_tensor("masks", [128, NSLOT * 2 * 128], BF16, kind="ExternalInput")
    d["out_own"] = nc.dram_tensor("out_own", [ROWS, C], F32, kind="ExternalOutput")

    with tile.TileContext(nc) as tc:
        _emit(nc, tc, d)
    nc.compile()
    return nc


def _emit(nc, tc, d):
    with ExitStack() as outer:
        # ---- small static tiles + LN scratch pools (whole kernel) ----
        stat = outer.enter_context(tc.tile_pool(name="static", bufs=1))
        ones = stat.tile([128, 128], BF16, tag="ones")        # row 0 = 1.0
        bqt = stat.tile([128, 8], F32, tag="bqt")
        bkt = stat.tile([128, 8], F32, tag="bkt")
        b1t = stat.tile([128, 32], F32, tag="b1t")
        nc.gpsimd.memset(ones[:], 0.0)
        nc.gpsimd.memset(ones[0:1, :], 1.0)
        nc.sync.dma_start(bqt[:], d["bq"].ap().rearrange("(a p) -> p a", p=128))
        nc.sync.dma_start(bkt[:], d["bk"].ap().rearrange("(a p) -> p a", p=128))
        nc.sync.dma_start(b1t[:], d["b1"].ap().rearrange("(a p) -> p a", p=128))

        pools = {}
        pools["stats"] = outer.enter_context(tc.tile_pool(name="stats", bufs=4))
        pools["sq"] = outer.enter_context(tc.tile_pool(name="sq", bufs=2))

        with ExitStack() as phab:  # spans phases A+B
            abp = phab.enter_context(tc.tile_pool(name="ab", bufs=1))
            kt = abp.tile([128, 8 * T], BF16, tag="kt")           # K^T  [kc8, T]
            qt = abp.tile([128, 8 * ROWS], BF16, tag="qt")        # Q^T  [qc8, ROWS]
            vp = abp.tile([128, NB * NH * 65], BF16, tag="vp")    # V'  [t16, h16, 65]
            msk = abp.tile([128, NSLOT * 2 * 128], BF16, tag="msk")
            kt3 = kt[:].rearrange("p (a t) -> p a t", a=8)
            qt3 = qt[:].rearrange("p (a t) -> p a t", a=8)
            vp4 = vp[:].rearrange("p (t h e) -> p t h e", t=NB, h=NH)
            msk3 = msk[:].rearrange("p (s q) -> p s q", s=NSLOT * 2)
            nc.gpsimd.memset(vp4[:, :, :, 64:65], 1.0)
            nc.sync.dma_start(msk[:], d["masks"].ap())

            # =============== Phase A: LN1 + Q/K/V projections ===============
            with ExitStack() as pha:
                wpool = pha.enter_context(tc.tile_pool(name="wqkv", bufs=1))
                wq_sb = wpool.tile([128, 8 * C], BF16, tag="wq")
                wk_sb = wpool.tile([128, 8 * C], BF16, tag="wk")
                wv_sb = wpool.tile([128, 8 * C], BF16, tag="wv")
                wq3 = wq_sb[:].rearrange("p (a c) -> p a c", a=8)
                wk3 = wk_sb[:].rearrange("p (a c) -> p a c", a=8)
                wv3 = wv_sb[:].rearrange("p (a c) -> p a c", a=8)
                nc.sync.dma_start(wq3, d["wq"].ap().rearrange("(a p) c -> p a c", p=128))
                nc.sync.dma_start(wk3, d["wk"].ap().rearrange("(a p) c -> p a c", p=128))
                nc.sync.dma_start(wv3, d["wv"].ap().rearrange("(a p) c -> p a c", p=128))

                htp = pha.enter_context(tc.tile_pool(name="ht", bufs=1))
                hts = []
                for i in range(4):
                    ht_i = htp.tile([128, 8 * 512], BF16, tag=f"ht{i}", name=f"ht{i}")
                    hts.append(ht_i)
                ht3s = [t[:].rearrange("p (a t) -> p a t", a=8) for t in hts]

                xpool = pha.enter_context(tc.tile_pool(name="xa", bufs=2))
                hpool = pha.enter_context(tc.tile_pool(name="hstage", bufs=3))
                pps = pha.enter_context(tc.tile_pool(name="ppsum", bufs=4, space="PSUM"))

                for chunk in range(4):
                    ht3 = ht3s[chunk]
                    for tt in range(4):
                        xa = xpool.tile([128, C], F32, tag="xa")
                        nc.sync.dma_start(
                            xa[:], d["x_full"].ap()[(chunk * 4 + tt) * 128:(chunk * 4 + tt + 1) * 128, :])
                        hst = hpool.tile([128, C], BF16, tag="h")
                        _ln_tile(nc, pools, xa[:], hst[:])
                        for cc in range(8):
                            nc.sync.dma_start_transpose(
                                ht3[:, cc, tt * 128:(tt + 1) * 128],
                                hst[:, cc * 128:(cc + 1) * 128])
                    # K projection for this chunk
                    for kc in range(8):
                        ps = pps.tile([128, 512], F32, tag="pp")
                        for cin in range(8):
                            nc.tensor.matmul(ps[:], wk3[:, cin, kc * 128:(kc + 1) * 128],
                                             ht3[:, cin, :], start=(cin == 0), stop=(cin == 7))
                        nc.scalar.activation(kt3[:, kc, chunk * 512:(chunk + 1) * 512], ps[:],
                                             AF.Identity, bias=bkt[:, kc:kc + 1])
                    # V projection (row-major)
                    for tt in range(4):
                        gt = chunk * 4 + tt
                        for hh in range(2):
                            ps = pps.tile([128, 512], F32, tag="pp")
                            for cin in range(8):
                                nc.tensor.matmul(ps[:], ht3[:, cin, tt * 128:(tt + 1) * 128],
                                                 wv3[:, cin, hh * 512:(hh + 1) * 512],
                                                 start=(cin == 0), stop=(cin == 7))
                            nc.scalar.activation(
                                vp4[:, gt, hh * 8:(hh + 1) * 8, 0:64],
                                ps[:].rearrange("p (h e) -> p h e", h=8), AF.Copy)
                _emit_q(nc, tc, pps, wq3, ht3s, qt3, bqt)

            # =============== Phase B: attention ===============
            # right-side pool for tensors that outlive phase B
            mid = phab.enter_context(tc.tile_pool(name="mid", bufs=1, side="right"))
            yt = mid.tile([128, 8 * ROWS], BF16, tag="yt")
            x2 = mid.tile([128, 8 * C], F32, tag="x2")
            h2t = mid.tile([128, 8 * ROWS], BF16, tag="h2t")
            yt3 = yt[:].rearrange("p (a t) -> p a t", a=8)
            x23 = x2[:].rearrange("p (a c) -> p a c", a=8)
            h2t3 = h2t[:].rearrange("p (a t) -> p a t", a=8)

            with ExitStack() as phb:
                spool = phb.enter_context(tc.tile_pool(name="spsum", bufs=3, space="PSUM"))
                ypsum = phb.enter_context(tc.tile_pool(name="ypsum", bufs=2, space="PSUM"))
                apool = phb.enter_context(tc.tile_pool(name="atile", bufs=3))
                ypool = phb.enter_context(tc.tile_pool(name="ysb", bufs=2))
                rpool = phb.enter_context(tc.tile_pool(name="rinv", bufs=4))

                for j in range(NSLOT):
                    tj = _trip(j)
                    y_sb = ypool.tile([128, C], BF16, tag="y")
                    for h in range(NH):
                        kth = kt3[64 * (h % 2):64 * (h % 2) + 64, h // 2, :]
                        qth = qt3[64 * (h % 2):64 * (h % 2) + 64, h // 2, j * 128:(j + 1) * 128]
                        py = ypsum.tile([128, 65], F32, tag="py")
                        ngrp = (tj + 3) // 4
                        for g in range(ngrp):
                            w = min(4, tj - g * 4)
                            ps = spool.tile([128, 512], F32, tag="ss")
                            for kk in range(w):
                                kb = g * 4 + kk
                                nc.tensor.matmul(ps[:, kk * 128:(kk + 1) * 128],
                                                 kth[:, kb * 128:(kb + 1) * 128], qth,
                                                 start=True, stop=True)
                            ag = apool.tile([128, 512], BF16, tag="ag")
                            nc.scalar.activation(ag[:, 0:w * 128], ps[:, 0:w * 128],
                                                 AF.Exp, scale=0.125)
                            for kk in range(w):
                                kb = g * 4 + kk
                                if kb >= tj - 2:
                                    m = kb - (tj - 2)
                                    nc.vector.tensor_mul(
                                        ag[:, kk * 128:(kk + 1) * 128],
                                        ag[:, kk * 128:(kk + 1) * 128],
                                        msk3[:, 2 * j + m, :])
                                nc.tensor.matmul(py[:], ag[:, kk * 128:(kk + 1) * 128],
                                                 vp4[:, kb, h, :],
                                                 start=(kb == 0), stop=(kb == tj - 1))
                        rinv = rpool.tile([128, 1], F32, tag="r")
                        nc.vector.reciprocal(rinv[:], py[:, 64:65])
                        nc.scalar.activation(y_sb[:, h * 64:(h + 1) * 64], py[:, 0:64],
                                             AF.Copy, scale=rinv[:])
                    for cc in range(8):
                        nc.sync.dma_start_transpose(
                            yt3[:, cc, j * 128:(j + 1) * 128],
                            y_sb[:, cc * 128:(cc + 1) * 128])

        # =============== Phase C: out-proj + residual, LN2 ===============
        with ExitStack() as phc:
            wpool = phc.enter_context(tc.tile_pool(name="wo", bufs=1))
            wo_sb = wpool.tile([128, 9 * C], BF16, tag="wo")
            wo3 = wo_sb[:].rearrange("p (a c) -> p a c", a=9)
            nc.sync.dma_start(wo3, d["wo"].ap().rearrange("(a p) c -> p a c", p=128))
            xop = phc.enter_context(tc.tile_pool(name="xo", bufs=1))
            xo = xop.tile([128, 8 * C], F32, tag="xo")
            xo3 = xo[:].rearrange("p (a c) -> p a c", a=8)
            nc.sync.dma_start(xo3, d["x_own"].ap().rearrange("(a p) c -> p a c", p=128))
            pps = phc.enter_context(tc.tile_pool(name="opsum", bufs=4, space="PSUM"))
            hpool = phc.enter_context(tc.tile_pool(name="h2stage", bufs=3))

            for ts in range(8):
                for cc in range(2):
                    ps = pps.tile([128, 512], F32, tag="op")
                    for yc in range(8):
                        nc.tensor.matmul(ps[:], yt3[:, yc, ts * 128:(ts + 1) * 128],
                                         wo3[:, yc, cc * 512:(cc + 1) * 512],
                                         start=(yc == 0), stop=False)
                    nc.tensor.matmul(ps[:], ones[:],
                                     wo3[:, 8, cc * 512:(cc + 1) * 512],
                                     start=False, stop=True)
                    nc.vector.tensor_add(x23[:, ts, cc * 512:(cc + 1) * 512], ps[:],
                                         xo3[:, ts, cc * 512:(cc + 1) * 512])
                hst = hpool.tile([128, C], BF16, tag="h2")
                _ln_tile(nc, pools, x23[:, ts, :], hst[:])
                for cc in range(8):
                    nc.sync.dma_start_transpose(
                        h2t3[:, cc, ts * 128:(ts + 1) * 128],
                        hst[:, cc * 128:(cc + 1) * 128])

        # =============== Phase D: FFN ===============
        with ExitStack() as phd:
            atp = phd.enter_context(tc.tile_pool(name="at", bufs=1))
            at = atp.tile([128, 33 * ROWS], BF16, tag="at")
            at3 = at[:].rearrange("p (f t) -> p f t", f=33)
            nc.gpsimd.memset(at3[:, 32, :], 0.0)
            nc.gpsimd.memset(at3[0:1, 32, :], 1.0)

            w1p = phd.enter_context(tc.tile_pool(name="w1s", bufs=24))
            pps = phd.enter_context(tc.tile_pool(name="fpsum", bufs=4, space="PSUM"))
            d_w1r = d["w1"].ap().rearrange("(a p) f -> p a f", p=128)
            for f in range(32):
                w1ts = []
                for cin in range(8):
                    w1t = w1p.tile([128, 128], BF16, tag="w1t")
                    nc.sync.dma_start(w1t[:], d_w1r[:, cin, f * 128:(f + 1) * 128])
                    w1ts.append(w1t)
                for chunk in range(2):
                    ps = pps.tile([128, 512], F32, tag="fp")
                    for cin in range(8):
                        nc.tensor.matmul(ps[:], w1ts[cin][:],
                                         h2t3[:, cin, chunk * 512:(chunk + 1) * 512],
                                         start=(cin == 0), stop=(cin == 7))
                    nc.scalar.activation(at3[:, f, chunk * 512:(chunk + 1) * 512], ps[:],
                                         AF.Relu, bias=b1t[:, f:f + 1])

            w2p = phd.enter_context(tc.tile_pool(name="w2s", bufs=36))
            outp = phd.enter_context(tc.tile_pool(name="outs", bufs=3))
            d_w2r = d["w2"].ap().rearrange("(a p) c -> p a c", p=128)
            for cc in range(2):
                w2ts = []
                for f in range(33):
                    w2t = w2p.tile([128, 512], BF16, tag="w2t")
                    nc.sync.dma_start(w2t[:], d_w2r[:, f, cc * 512:(cc + 1) * 512])
                    w2ts.append(w2t)
                for ts in range(8):
                    ps = pps.tile([128, 512], F32, tag="fp")
                    for f in range(33):
                        nc.tensor.matmul(ps[:], at3[:, f, ts * 128:(ts + 1) * 128],
                                         w2ts[f][:], start=(f == 0), stop=(f == 32))
                    ot = outp.tile([128, 512], F32, tag="ot")
                    nc.vector.tensor_add(ot[:], ps[:], x23[:, ts, cc * 512:(cc + 1) * 512])
                    nc.sync.dma_start(
                        d["out_own"].ap()[ts * 128:(ts + 1) * 128, cc * 512:(cc + 1) * 512],
                        ot[:])


def _emit_q(nc, tc, pps, wq3, ht3s, qt3, bqt):
    """Q projection for the core's own q-blocks.

    The host permutes x_full's 128-row blocks per core so that the core's own
    q-blocks sit at EVEN permuted positions (see make_in_maps); the strided
    rhs below selects in-chunk positions {0, 2}, i.e. slots 2*chunk and
    2*chunk+1, for every core with one uniform access pattern.
    """
    for chunk in range(4):
        ht3 = ht3s[chunk]
        for qc in range(8):
            ps = pps.tile([128, 512], F32, tag="pp")
            for cin in range(8):
                rr = ht3[:, cin, :].rearrange("p (s d t) -> p s d t", s=2, d=2)[:, :, 0, :]
                nc.tensor.matmul(ps[:, 0:256], wq3[:, cin, qc * 128:(qc + 1) * 128],
                                 rr, start=(cin == 0), stop=(cin == 7))
            nc.scalar.activation(qt3[:, qc, chunk * 256:(chunk + 1) * 256], ps[:, 0:256],
                                 AF.Identity, bias=bqt[:, qc:qc + 1])


# ---------------------------------------------------------------- host side

_NC_CACHE = None


def _get_nc():
    global _NC_CACHE
    if _NC_CACHE is None:
        _NC_CACHE = build_program()
    return _NC_CACHE


def _bf16(a):
    return np.asarray(a, dtype=np.float32).astype(ml_dtypes.bfloat16)


def make_in_maps(x, Wq, Wk, Wv, Wo, bo, W1, b1, W2, b2, g1, be1, g2, be2):
    x = np.asarray(x, dtype=np.float32)
    g1 = np.asarray(g1, np.float32); be1 = np.asarray(be1, np.float32)
    g2 = np.asarray(g2, np.float32); be2 = np.asarray(be2, np.float32)
    Wq = np.asarray(Wq, np.float32); Wk = np.asarray(Wk, np.float32)
    Wv = np.asarray(Wv, np.float32); Wo = np.asarray(Wo, np.float32)
    W1 = np.asarray(W1, np.float32); W2 = np.asarray(W2, np.float32)
    bo = np.asarray(bo, np.float32); b1 = np.asarray(b1, np.float32)
    b2 = np.asarray(b2, np.float32)

    wq_e = _bf16(g1[:, None] * Wq)
    wk_e = _bf16(g1[:, None] * Wk)
    wv_e = _bf16(g1[:, None] * Wv)
    bq = (be1 @ Wq).astype(np.float32)
    bk = (be1 @ Wk).astype(np.float32)
    bv = (be1 @ Wv).astype(np.float32)
    # softmax rows sum to 1 => y_h = (sm @ V_h) + bv_h; fold bv@Wo into bo.
    bo_eff = (bo + bv @ Wo).astype(np.float32)
    wo_pad = np.zeros((C + 128, C), np.float32)
    wo_pad[:C] = Wo
    wo_pad[C] = bo_eff
    wo_pad = _bf16(wo_pad)
    w1_e = _bf16(g2[:, None] * W1)
    b1v = (be2 @ W1 + b1).astype(np.float32)
    w2_pad = np.zeros((FF + 128, C), np.float32)
    w2_pad[:FF] = W2
    w2_pad[FF] = b2
    w2_pad = _bf16(w2_pad)

    tri = np.triu(np.ones((128, 128), np.float32))  # [k, q]: keep k <= q
    in_maps = []
    for core in range(NCORES):
        b, half = core // 2, core % 2
        own = _own_blocks(half)
        other = _own_blocks(1 - half)
        # permuted block order: own blocks at even positions
        perm = []
        for j in range(NSLOT):
            perm.append(own[j])
            perm.append(other[j])
        # perm[p] = original block at permuted position p
        x_perm = np.concatenate([x[b, g * 128:(g + 1) * 128, :] for g in perm], axis=0)
        x_own = np.concatenate([x[b, g * 128:(g + 1) * 128, :] for g in own], axis=0)
        # masks: slot j (own block g=own[j], orig row range [128g, 128g+128))
        # attends permuted key blocks 0..Tj-1; mask on the last two.
        masks = np.zeros((NSLOT, 2, 128, 128), np.float32)
        for j in range(NSLOT):
            tj = _trip(j)
            g = own[j]
            q_orig = g * 128 + np.arange(128)          # original query rows
            for m in range(2):
                kb = tj - 2 + m                        # permuted key block idx
                k_orig = perm[kb] * 128 + np.arange(128)
                masks[j, m] = (k_orig[:, None] <= q_orig[None, :]).astype(np.float32)
        masks_t = _bf16(np.transpose(masks, (2, 0, 1, 3)).reshape(128, NSLOT * 2 * 128))
        in_maps.append({
            "x_full": np.ascontiguousarray(x_perm),
            "x_own": np.ascontiguousarray(x_own),
            "wq": wq_e, "wk": wk_e, "wv": wv_e, "wo": wo_pad,
            "w1": w1_e, "w2": w2_pad,
            "bq": bq, "bk": bk, "b1": b1v,
            "masks": masks_t,
        })
    return in_maps


def scatter_out(results):
    out = np.empty((B, T, C), np.float32)
    for core in range(NCORES):
        b, half = core // 2, core % 2
        own = _own_blocks(half)
        oo = results[core]["out_own"]
        for j, g in enumerate(own):
            out[b, g * 128:(g + 1) * 128, :] = oo[j * 128:(j + 1) * 128, :]
    return out


def kernel(**inputs):
    nc = _get_nc()
    in_maps = make_in_maps(**inputs)
    res = bass_utils.run_bass_kernel_spmd(nc, in_maps, core_ids=list(range(NCORES)))
    return scatter_out(res.results)
